# revision 1
# baseline (speedup 1.0000x reference)
"""Masked phase-locking value (PLV) kernel for Trainium2, 8 NeuronCores.

Math: out[b] = |sum_ij M_ij * exp(i*(a_bi - b_bj))| / max(sum(M), 1)
    real_b = sum_ij M_ij (cos a_bi cos b_bj + sin a_bi sin b_bj)
    imag_b = sum_ij M_ij (sin a_bi cos b_bj - cos a_bi sin b_bj)

Device decomposition (per core, Na sharded 8 ways -> NI=1024 rows each):
    Y[m, i] = sum_j V[j, m] * M[i, j]      (TensorE, fp8 DoubleRow; V = [cb^T | sb^T], m = 2B = 128)
    racc[m] = sum_i Y[m, i] * U[m, i]      (DVE fused scalar_tensor_tensor; U = [ca;sa] / [sa;-ca])
real_b = sum_cores racc_r[b] + racc_r[64+b]; imag_b likewise from racc_q.
Bilinear in mask rows, so Na-shard partials just add; host does the tiny
fold + |z| / sum(M).

The kernel is HBM-bound: ~9.25MB/core (8MB mask fp8 + 1MB trig weights fp8
+ 0.25MB U fp8) at the ~330GB/s per-core share of chip HBM. Everything else
hides under the mask stream:
  - matmul flipped vs naive (trig stationary, mask moving): PSUM output is
    [128, 1024] instead of [128, 8192] -> 8x smaller epilogue.
  - fp8 DoubleRow matmul (contraction 256/instr, FD=512): ~1.5x PE rate;
    mask is 0/1 -> exact in fp8; trig fp8 (end-to-end rel err ~3e-3 << 2e-2).
  - each matmul accumulation region owns a full PSUM bank: start_tensor_calc
    zeroes the whole bank row, so regions must never share a bank.
  - mask streamed on the sync HWDGE ring as few large blocks (descriptor-gen
    and per-transfer completion latency amortized), tiny last block so the
    final completion semaphore gates almost no work; trig on the scalar ring.
  - PE warm-up on memset junk (no DMA dependency) defeats the cold-clock ramp.
  - epilogue (one fused DVE op per i-slice half) fires per bank as soon as
    that bank's accumulation closes; first results DMA out under the tail.
"""

import numpy as np

import concourse.bass as bass
import concourse.tile as tile
from concourse import bacc, mybir
from concourse.bass_utils import run_bass_kernel_spmd

B = 64
NA = 8192
NB = 8192
NCORES = 8
NI = NA // NCORES            # mask rows (i) per core
KC = NB // 256               # 32 contraction chunks of 256 j-rows
TK = 2 * KC                  # tile dim1: t = 2k + q (DoubleRow pair slot q)
MMSL = 256                   # matmul i-slice (FD); each owns a full PSUM bank
NBK = NI // MMSL             # 4 accumulation banks
ESL = 256                    # epilogue i-slice
NSL = NI // ESL              # 4 epilogue slices
NWU = 12                     # PE warm-up matmuls
# mask DMA blocks in k-chunks (256KB each), alternating between the two
# HWDGE rings: ~1MB completion-sem granularity keeps the PE fed while each
# ring's transfer boundary hides under the other ring's stream; tiny last
# block so the final completion sem gates minimal work
MBLK_K = [(4, 1), (8, 1), (8, 1), (8, 1), (3, 1), (1, 1)]
assert sum(nk for nk, _ in MBLK_K) == KC

F8 = mybir.dt.float8e4
F16 = mybir.dt.float16
F32 = mybir.dt.float32


def build_program() -> bass.Bass:
    nc = bacc.Bacc("TRN2")
    # host layouts are p-major: dim0 = SBUF partition, per-partition contiguous
    mask_d = nc.dram_tensor("mask", [128, TK, NI], F8, kind="ExternalInput")
    v_d = nc.dram_tensor("v", [128, TK, 2 * B], F8, kind="ExternalInput")
    u_d = nc.dram_tensor("u", [128, 2, NI], F8, kind="ExternalInput")
    out_d = nc.dram_tensor("out", [128, 2 * NBK], F32, kind="ExternalOutput")

    DR = mybir.MatmulPerfMode.DoubleRow
    mult = mybir.AluOpType.mult

    with tile.TileContext(nc) as tc:
        with (
            tc.tile_pool(name="consts", bufs=1) as consts,
            tc.tile_pool(name="psum", bufs=1, space="PSUM") as psum_pool,
        ):
            jw = consts.tile([128, 2, MMSL], F8)
            nc.vector.memset(jw, 0)
            mask_sb = consts.tile([128, TK, NI], F8)
            v_sb = consts.tile([128, TK, 2 * B], F8)
            u_sb = consts.tile([128, 2, NBK, ESL], F8)
            racc = consts.tile([128, 2 * NBK], F32)
            jr = consts.tile([128, ESL], F32)

            # V (gates k=0) and U ride the scalar ring; the mask stream owns
            # the sync ring uninterrupted
            nc.scalar.dma_start(out=v_sb[:], in_=v_d[:])
            nc.scalar.dma_start(out=u_sb[:], in_=u_d[:])
            k0 = 0
            for nk, _ in MBLK_K:
                tsl = slice(2 * k0, 2 * (k0 + nk))
                nc.sync.dma_start(out=mask_sb[:, tsl], in_=mask_d[:, tsl])
                k0 += nk

            # one accumulation region per PSUM bank (start_tensor_calc
            # zeroes the whole bank row), as SEPARATE tiles: a merged tile
            # makes the tile framework serialize each bank's final matmul
            # behind the previous bank's epilogue reads
            pss = [
                psum_pool.tile([128, 512], F32, name=f"ps{i}") for i in range(NBK)
            ]
            wu = psum_pool.tile([128, 512], F32)

            # PE warm-up on junk (no DMA dependency) to beat the clock ramp
            for r in range(NWU):
                nc.tensor.matmul(
                    out=wu[:, 0:MMSL], lhsT=jw[:, :, 0:128], rhs=jw[:],
                    start=(r == 0), stop=(r == NWU - 1), perf_mode=DR,
                )

            for k in range(KC):
                tsl = slice(2 * k, 2 * k + 2)
                for sb in range(NBK):
                    msl = slice(sb * MMSL, (sb + 1) * MMSL)
                    nc.tensor.matmul(
                        out=pss[sb][:, 0:MMSL],
                        lhsT=v_sb[:, tsl, :],
                        rhs=mask_sb[:, tsl, msl],
                        start=(k == 0), stop=(k == KC - 1), perf_mode=DR,
                    )
                    if k == KC - 1:
                        # bank closed: fused multiply+reduce, overlapping
                        # the remaining banks' matmuls
                        for h in (0, 1):
                            col = 2 * sb + h
                            nc.vector.scalar_tensor_tensor(
                                out=jr[:], in0=pss[sb][:, 0:MMSL],
                                scalar=1.0, in1=u_sb[:, h, sb],
                                op0=mult, op1=mult,
                                accum_out=racc[:, col : col + 1],
                            )
                        if sb == 1:
                            # first half's results fly out under the tail
                            nc.sync.dma_start(
                                out=out_d[:, : NBK], in_=racc[:, : NBK]
                            )
            nc.sync.dma_start(out=out_d[:, NBK:], in_=racc[:, NBK:])
    nc.finalize()
    return nc


def prep_inputs(phases_a, phases_b, coupling_mask):
    f8np = mybir.dt.np(F8)
    pa = np.asarray(phases_a, dtype=np.float32)
    pb = np.asarray(phases_b, dtype=np.float32)
    ca, sa = np.cos(pa), np.sin(pa)
    cb, sb = np.cos(pb), np.sin(pb)

    one_byte = np.array([1.0], f8np).view(np.uint8)[0]
    mask_u8 = (np.asarray(coupling_mask) != 0).astype(np.uint8) * one_byte

    # V[p, t=2k+q, m]: trig value for j = 256k + 2p + q, m = batch (cb|sb)
    T2 = np.concatenate([cb, sb], axis=0)                      # [128 m, NB j]
    v_host = (
        np.ascontiguousarray(T2.T.reshape(KC, 128, 2, 2 * B).transpose(1, 0, 2, 3))
        .reshape(128, TK, 2 * B)
        .astype(f8np)
    )

    in_maps = []
    for c in range(NCORES):
        sl = slice(c * NI, (c + 1) * NI)
        A = mask_u8[sl]                                        # [NI i, NB j]
        m_host = (
            np.ascontiguousarray(A.reshape(NI, KC, 128, 2).transpose(2, 1, 3, 0))
            .reshape(128, TK, NI)
            .view(f8np)
        )
        u_host = np.stack(
            [
                np.concatenate([ca[:, sl], sa[:, sl]], axis=0),
                np.concatenate([sa[:, sl], -ca[:, sl]], axis=0),
            ],
            axis=1,
        ).astype(f8np)                                         # [128, 2, NI]
        in_maps.append({"mask": m_host, "v": v_host, "u": u_host})
    return in_maps


def combine(outs, coupling_mask):
    o = np.stack(outs).astype(np.float64)      # [NCORES, 128, 2*NBK]
    r = o[:, :, 0::2].sum(axis=(0, 2))         # [128]
    q = o[:, :, 1::2].sum(axis=(0, 2))
    real = r[:B] + r[B:]
    imag = q[:B] + q[B:]
    n_pairs = max(float(np.count_nonzero(np.asarray(coupling_mask))), 1.0)
    return (np.sqrt(real * real + imag * imag) / n_pairs).astype(np.float32)


_prog_cache: list = []


def kernel(phases_a, phases_b, coupling_mask):
    in_maps = prep_inputs(phases_a, phases_b, coupling_mask)
    if not _prog_cache:
        _prog_cache.append(build_program())
    res = run_bass_kernel_spmd(_prog_cache[0], in_maps, core_ids=list(range(NCORES)))
    return combine([r["out"] for r in res.results], coupling_mask)



# revision 2
# speedup vs baseline: 1.1546x; 1.1546x over previous
"""Masked phase-locking value (PLV) kernel for Trainium2, 8 NeuronCores.

Math: out[b] = |sum_ij M_ij * exp(i*(a_bi - b_bj))| / max(sum(M), 1)
    real_b = sum_ij M_ij (cos a_bi cos b_bj + sin a_bi sin b_bj)
    imag_b = sum_ij M_ij (sin a_bi cos b_bj - cos a_bi sin b_bj)

Device decomposition (per core, Na sharded 8 ways -> NI=1024 rows each):
    Y[m, i] = sum_j V[j, m] * M[i, j]      (TensorE; V = [cb^T | sb^T], m = 2B = 128)
    racc[m] = sum_i Y[m, i] * U[m, i]      (DVE fused scalar_tensor_tensor)

The mask is binary, so 4 mask elements ride in each fp8 byte as BIT PLANES
(bits 0x08/0x10/0x20/0x40 = exact fp8e4 values 2^-6/2^-5/2^-3/2.0):
  - HBM mask traffic drops 8MB -> 2MB per core; the whole stream is
    2MB mask + 1MB trig weights + 0.25MB U = 3.25MB (~9us at 358GB/s).
  - on-device extraction = one DVE tensor_scalar(bitwise_and) per
    (chunk, plane) on uint32-punned data -> byte-exact fp8 plane tensors.
    uint32 runs in 2x_2P mode (8 bytes/cycle/lane): ~1.55us/chunk, under
    the PE's 1.74us/chunk consumption rate.
  - per-plane scale 2^k folds into the fp8 weights (|w| <= 64 < 240 max),
    so precision is identical to the unpacked fp8 baseline.
The PE runs 8 chunks x 4 planes x 4 banks = 128 DoubleRow matmuls
(contraction 256/instr, FD=256) = the same 13.7us fp8 roofline as the
unpacked kernel, but now it IS the pacing engine instead of the DMA.

Schedule: sync ring carries v[kb0], pk[kb0], then the rest of the packed
mask; scalar ring carries the remaining weights + U. 18 warm-up matmuls
(~3.8us > the 3.4us HAM window) bring the PE clock to 2.4GHz before the
first real matmul. The last chunk runs plane-major with per-bank closes:
each bank's epilogue STT pair fires as soon as its stop-matmul retires,
and racc[:, :6] flies out right after bank 2 so only bank 3's epilogue
and a 64B DMA sit on the tail.
"""

import numpy as np

import concourse.bass as bass
import concourse.tile as tile
from concourse import bacc, mybir
from concourse.bass_utils import run_bass_kernel_spmd

B = 64
NA = 8192
NB = 8192
NCORES = 8
NI = NA // NCORES            # mask rows (i) per core
NPL = 4                      # mask bit-planes packed per byte
NJB = NB // NPL              # 2048 packed bytes per mask row
KC = NJB // 256              # 8 contraction chunks of 256 bytes
TK = 2 * KC                  # tile dim1: t = 2*kb + q (DoubleRow pair slot q)
NIW = NI // 4                # uint32 words per (partition, t) row
MMSL = 256                   # matmul i-slice (FD); each bank owns a full PSUM bank
NBK = NI // MMSL             # 4 accumulation banks
NWU = 18                     # PE warm-up matmuls (>3.4us busy -> HAM warm)
BITS = [0x08, 0x10, 0x20, 0x40]
BITVAL = [2.0 ** -6, 2.0 ** -5, 2.0 ** -3, 2.0]
ANDMASK = [b * 0x01010101 for b in BITS]

F8 = mybir.dt.float8e4
U32 = mybir.dt.uint32
F32 = mybir.dt.float32


def build_program() -> bass.Bass:
    nc = bacc.Bacc("TRN2")
    # host layouts are p-major: dim0 = SBUF partition, per-partition contiguous
    pk_d = nc.dram_tensor("pk", [128, TK, NIW], U32, kind="ExternalInput")
    v_d = nc.dram_tensor("v", [128, KC, NPL, 2, 2 * B], F8, kind="ExternalInput")
    u_d = nc.dram_tensor("u", [128, 2, NI], F8, kind="ExternalInput")
    out_d = nc.dram_tensor("out", [128, 2 * NBK], F32, kind="ExternalOutput")

    DR = mybir.MatmulPerfMode.DoubleRow
    mult = mybir.AluOpType.mult
    band = mybir.AluOpType.bitwise_and

    with tile.TileContext(nc) as tc:
        with (
            tc.tile_pool(name="consts", bufs=1) as consts,
            tc.tile_pool(name="psum", bufs=1, space="PSUM") as psum_pool,
        ):
            jw = consts.tile([128, 2, MMSL], F8)
            nc.vector.memset(jw, 0)
            pk_sb = consts.tile([128, TK, NIW], U32)
            pl_sb = consts.tile([128, NPL, TK, NIW], U32)
            v_sb = consts.tile([128, KC, NPL, 2, 2 * B], F8)
            u_sb = consts.tile([128, 2, NBK, MMSL], F8)
            racc = consts.tile([128, 2 * NBK], F32)
            jr = consts.tile([128, MMSL], F32)

            # DMA plan, in consumption order. The sync ring carries the first
            # chunk's weights + packed mask so the PE can start ASAP; the
            # scalar ring streams the remaining weights and U in parallel.
            nc.sync.dma_start(out=v_sb[:, 0:1], in_=v_d[:, 0:1])
            nc.sync.dma_start(out=pk_sb[:, 0:2], in_=pk_d[:, 0:2])
            nc.sync.dma_start(out=pk_sb[:, 2:8], in_=pk_d[:, 2:8])
            nc.sync.dma_start(out=pk_sb[:, 8:14], in_=pk_d[:, 8:14])
            nc.sync.dma_start(out=pk_sb[:, 14:16], in_=pk_d[:, 14:16])
            nc.scalar.dma_start(out=v_sb[:, 1:3], in_=v_d[:, 1:3])
            nc.scalar.dma_start(out=v_sb[:, 3:8], in_=v_d[:, 3:8])
            nc.scalar.dma_start(out=u_sb[:], in_=u_d[:])

            # one accumulation region per PSUM bank (start_tensor_calc
            # zeroes the whole bank row), as SEPARATE tiles: a merged tile
            # makes the tile framework serialize each bank's final matmul
            # behind the previous bank's epilogue reads
            pss = [
                psum_pool.tile([128, 512], F32, name=f"ps{i}") for i in range(NBK)
            ]
            wu = psum_pool.tile([128, 512], F32)

            # PE warm-up on junk (no DMA dependency) to beat the clock ramp
            for r in range(NWU):
                nc.tensor.matmul(
                    out=wu[:, 0:MMSL], lhsT=jw[:, :, 0:128], rhs=jw[:],
                    start=(r == 0), stop=(r == NWU - 1), perf_mode=DR,
                )

            # plane extraction: bitwise AND on uint32-punned bytes; emitted
            # in consumption order so the DVE FIFO matches the PE's needs
            for kb in range(KC):
                tsl = slice(2 * kb, 2 * kb + 2)
                for k in range(NPL):
                    nc.vector.tensor_scalar(
                        out=pl_sb[:, k, tsl], in0=pk_sb[:, tsl],
                        scalar1=ANDMASK[k], scalar2=None, op0=band,
                    )

            def rhs(kb, k, sb):
                return pl_sb[
                    :, k, 2 * kb : 2 * kb + 2, 64 * sb : 64 * (sb + 1)
                ].bitcast(F8)

            for kb in range(KC):
                for k in range(NPL):
                    lhsT = v_sb[:, kb, k]
                    for sb in range(NBK):
                        nc.tensor.matmul(
                            out=pss[sb][:, 0:MMSL],
                            lhsT=lhsT,
                            rhs=rhs(kb, k, sb),
                            start=(kb == 0 and k == 0),
                            stop=(kb == KC - 1 and k == NPL - 1),
                            perf_mode=DR,
                        )
                        if kb == KC - 1 and k == NPL - 1:
                            # bank closed: fused multiply+reduce fires as
                            # soon as this bank's stop-matmul retires,
                            # overlapping the remaining banks' matmuls
                            for h in (0, 1):
                                col = 2 * sb + h
                                nc.vector.scalar_tensor_tensor(
                                    out=jr[:], in0=pss[sb][:, 0:MMSL],
                                    scalar=1.0, in1=u_sb[:, h, sb],
                                    op0=mult, op1=mult,
                                    accum_out=racc[:, col : col + 1],
                                )
                            if sb == NBK - 2:
                                # first three banks' results fly out early
                                nc.sync.dma_start(
                                    out=out_d[:, : 2 * (NBK - 1)],
                                    in_=racc[:, : 2 * (NBK - 1)],
                                )
            nc.sync.dma_start(
                out=out_d[:, 2 * (NBK - 1) :], in_=racc[:, 2 * (NBK - 1) :]
            )
    nc.finalize()
    return nc


def prep_inputs(phases_a, phases_b, coupling_mask):
    f8np = mybir.dt.np(F8)
    pa = np.asarray(phases_a, dtype=np.float32)
    pb = np.asarray(phases_b, dtype=np.float32)
    ca, sa = np.cos(pa), np.sin(pa)
    cb, sb = np.cos(pb), np.sin(pb)

    m_u8 = (np.asarray(coupling_mask) != 0).astype(np.uint8)

    # weights: V[p, kb, k, q, m] = T2[m, j]/BITVAL[k], j = 4*(256kb+2p+q)+k
    T2 = np.concatenate([cb, sb], axis=0)                      # [128 m, NB j]
    W = np.ascontiguousarray(T2.T)                             # [NB j, 128 m]
    W = W.reshape(KC, 128, 2, NPL, 128).transpose(1, 0, 3, 2, 4)
    W = W / np.asarray(BITVAL, np.float32)[None, None, :, None, None]
    v_host = W.astype(f8np)                                    # [128,KC,NPL,2,128]

    in_maps = []
    for c in range(NCORES):
        sl = slice(c * NI, (c + 1) * NI)
        A = m_u8[sl]                                           # [NI i, NB j]
        # pack 4 j's per byte at bits 3..6: byte[i, jb] = sum_k A[i,4jb+k]<<(3+k)
        A4 = A.reshape(NI, NJB, NPL)
        P = (
            (A4[:, :, 0] << 3) | (A4[:, :, 1] << 4)
            | (A4[:, :, 2] << 5) | (A4[:, :, 3] << 6)
        ).astype(np.uint8)                                     # [NI, NJB]
        pk_host = (
            np.ascontiguousarray(P.reshape(NI, KC, 128, 2).transpose(2, 1, 3, 0))
            .reshape(128, TK, NI)
            .view(np.uint32)
        )                                                      # [128, TK, NIW]
        u_host = np.stack(
            [
                np.concatenate([ca[:, sl], sa[:, sl]], axis=0),
                np.concatenate([sa[:, sl], -ca[:, sl]], axis=0),
            ],
            axis=1,
        ).astype(f8np)                                         # [128, 2, NI]
        in_maps.append({"pk": pk_host, "v": v_host, "u": u_host})
    return in_maps


def combine(outs, coupling_mask):
    o = np.stack(outs).astype(np.float64)      # [NCORES, 128, 2*NBK]
    r = o[:, :, 0::2].sum(axis=(0, 2))         # [128]
    q = o[:, :, 1::2].sum(axis=(0, 2))
    real = r[:B] + r[B:]
    imag = q[:B] + q[B:]
    n_pairs = max(float(np.count_nonzero(np.asarray(coupling_mask))), 1.0)
    return (np.sqrt(real * real + imag * imag) / n_pairs).astype(np.float32)


_prog_cache: list = []


def kernel(phases_a, phases_b, coupling_mask):
    in_maps = prep_inputs(phases_a, phases_b, coupling_mask)
    if not _prog_cache:
        _prog_cache.append(build_program())
    res = run_bass_kernel_spmd(_prog_cache[0], in_maps, core_ids=list(range(NCORES)))
    return combine([r["out"] for r in res.results], coupling_mask)
